# revision 1
# baseline (speedup 1.0000x reference)
"""CosineAttention Trainium2 Bass kernel.

Computes softmax(cos_sim(keys[b,l,:], query[b,:]) masked) over l, for
B=64, L=4096, D=1024, sharded batch-parallel over 8 NeuronCores
(8 batches per core, 128 MiB of keys per core -> memory bound).

Math per (b, l):
    dot[l]  = sum_d keys[b,l,d] * qhat[b,d]      (qhat = q / max(||q||, eps))
    ssq[l]  = sum_d keys[b,l,d]^2
    score   = dot / max(sqrt(ssq), eps) + (mask-1)*1e30
    out     = exp(score) / sum_l exp(score)      (scores in [-1,1]: no max-sub needed)

Engine plan per core:
  - DMA   : stream keys in 4 MiB chunks (contiguous 32 KiB per partition)
  - DVE   : fused tensor_tensor_reduce (mult + add-reduce) -> dot   (1 pass)
  - ACT   : fused activation(Square, accum_out=)           -> ssq   (1 pass)
  - PE    : ones-matmul for the cross-partition softmax denominator
L is laid out interleaved: l = p*T + t (p = partition, T = L/128), so both
the keys loads and the output store are contiguous per partition.
"""

import numpy as np

import concourse.bass as bass
import concourse.tile as tile
from concourse import bacc, mybir

P = 128          # SBUF partitions
B = 64           # full batch
L = 4096
D = 1024
N_CORES = 8
BPC = B // N_CORES   # batches per core
CJ = 8               # l-tiles per DMA chunk (4 MiB chunks)

F32 = mybir.dt.float32
U8 = mybir.dt.uint8
Alu = mybir.AluOpType
Act = mybir.ActivationFunctionType

EPS = 1e-12
NEG_BIG = 1.0e30


def build_nc(bpc=BPC, l_dim=L, d=D, cj=CJ, n_cores=N_CORES, reps=1,
             variant="full", kbufs=3, dma_eng="sync", dma_split=1, loop_n=0,
             fold_qnorm=True, epi="batch"):
    do_dve = variant in ("full", "dma_dve")
    do_act = variant in ("full", "dma_act")
    t_cols = l_dim // P       # score columns per partition
    nch = t_cols // cj        # chunks per batch
    assert t_cols * P == l_dim and nch * cj == t_cols

    nc = bacc.Bacc(
        "TRN2",
        target_bir_lowering=False,
        debug=False,
        enable_asserts=False,
        num_devices=n_cores,
    )

    q_t = nc.dram_tensor("q", [bpc, d], F32, kind="ExternalInput")
    keys_t = nc.dram_tensor("keys", [bpc, l_dim, d], F32, kind="ExternalInput")
    mask_t = nc.dram_tensor("mask", [bpc, l_dim], U8, kind="ExternalInput")
    out_t = nc.dram_tensor("out", [bpc, l_dim], F32, kind="ExternalOutput")

    q_ap = q_t.ap()
    keys_ap = keys_t.ap()
    mask_ap = mask_t.ap()
    out_ap = out_t.ap()

    with tile.TileContext(nc) as tc:
        with (
            tc.tile_pool(name="kpool", bufs=kbufs) as kpool,
            tc.tile_pool(name="singles", bufs=1) as singles,
            tc.tile_pool(name="ascr", bufs=2) as ascr,
            tc.tile_pool(name="psum", bufs=2, space="PSUM") as psum,
        ):
            # --- persistent tiles ---
            qrep = singles.tile([P, bpc, d], F32)        # q replicated to all partitions
            maskf = singles.tile([P, bpc * t_cols], F32) # mask -> additive bias
            qss = singles.tile([P, bpc], F32)            # per-batch ||q||^2
            ones = singles.tile([P, P], F32)             # for cross-partition sum matmul
            negbig = singles.tile([P, 1], F32)           # bias tile for mask rescale

            vdummy = singles.tile([P, 1], F32)           # step-0 sink for fused dot

            nc.vector.memset(ones, 1.0)
            nc.vector.memset(negbig, -NEG_BIG)

            # Broadcast q to all 128 partitions during the DMA (partition step 0).
            q_bcast = bass.AP(
                tensor=q_ap.tensor,
                offset=q_ap.offset,
                ap=[[0, P], [d, bpc], [1, d]],
            )
            nc.gpsimd.dma_start(out=qrep, in_=q_bcast)

            # Mask: u8 -> f32 cast during DMA.  DRAM layout per batch is
            # [P, t_cols] with l = p*t_cols + t.
            mask_v = mask_ap.rearrange("b (p t) -> p b t", p=P)
            nc.gpsimd.dma_start(
                out=maskf[:].rearrange("p (b t) -> p b t", b=bpc), in_=mask_v
            )

            # mask -> additive bias {0, -1e30}, done once up front
            nc.scalar.activation(out=maskf, in_=maskf, func=Act.Identity,
                                 bias=negbig[:, 0:1], scale=NEG_BIG)

            # --- q norms (per-partition identical values) ---
            for b in range(bpc):
                s = ascr.tile([P, d], F32)
                nc.scalar.activation(out=s, in_=qrep[:, b, :], func=Act.Square,
                                     accum_out=qss[:, b : b + 1])
            nc.scalar.activation(out=qss, in_=qss, func=Act.Sqrt)
            nc.vector.tensor_scalar_max(qss, qss, EPS)
            nc.vector.reciprocal(qss, qss)            # 1/||q|| per batch
            if not fold_qnorm:
                # normalize q up front (1/||q|| folded into epilogue otherwise)
                for b in range(bpc):
                    nc.vector.tensor_scalar_mul(qrep[:, b, :], qrep[:, b, :],
                                                qss[:, b : b + 1])

            import contextlib

            loop_cm = tc.For_i(0, loop_n, 1) if loop_n else contextlib.nullcontext()
            with loop_cm:
              for _rep in range(reps):
                # per-rep accumulators (bufs=1 tags -> reps serialize on slots)
                dots = singles.tile([P, bpc * t_cols], F32, tag="dots")
                ssqs = singles.tile([P, bpc * t_cols], F32, tag="ssqs")
                if not do_dve:
                    nc.vector.memset(dots, 0.0)
                if not do_act:
                    nc.vector.memset(ssqs, 1.0)

                # --- epilogue emitter: normalize scores, mask, softmax, store
                den = singles.tile([P, bpc], F32, tag="den")
                out_v = out_ap.rearrange("b (p t) -> p b t", p=P)

                def emit_epilogue(b, dots=None, ssqs=None):
                    dots, ssqs = dots or _acc[0], ssqs or _acc[1]
                    sl = slice(b * t_cols, (b + 1) * t_cols)
                    nc.scalar.activation(out=ssqs[:, sl], in_=ssqs[:, sl],
                                         func=Act.Sqrt)           # ||k||
                    nc.vector.tensor_scalar_max(ssqs[:, sl], ssqs[:, sl], EPS)
                    nc.vector.reciprocal(ssqs[:, sl], ssqs[:, sl])  # 1/||k||
                    nc.vector.tensor_mul(dots[:, sl], dots[:, sl], ssqs[:, sl])
                    if fold_qnorm:
                        nc.vector.tensor_scalar_mul(dots[:, sl], dots[:, sl],
                                                    qss[:, b : b + 1])
                    nc.vector.tensor_add(dots[:, sl], dots[:, sl], maskf[:, sl])
                    nc.scalar.activation(out=dots[:, sl], in_=dots[:, sl],
                                         func=Act.Exp)
                    # denominator: ones.T @ E sums across partitions; then
                    # reduce the t_cols columns; every partition ends up
                    # with the full sum.
                    mm = psum.tile([P, t_cols], F32, tag="mm")
                    nc.tensor.matmul(out=mm, lhsT=ones, rhs=dots[:, sl],
                                     start=True, stop=True)
                    nc.vector.tensor_reduce(out=den[:, b : b + 1], in_=mm,
                                            axis=mybir.AxisListType.X,
                                            op=Alu.add)
                    nc.vector.reciprocal(den[:, b : b + 1], den[:, b : b + 1])
                    nc.vector.tensor_scalar_mul(dots[:, sl], dots[:, sl],
                                                den[:, b : b + 1])
                    nc.sync.dma_start(out=out_v[:, b, :], in_=dots[:, sl])

                _acc = (dots, ssqs)

                # --- main loop: stream keys, fused dot + ssq reductions ---
                for b in range(bpc):
                    kv = keys_ap[b].rearrange("(p c j) d -> p c (j d)", p=P, c=nch)
                    if epi == "batch" and b >= 1:
                        emit_epilogue(b - 1)
                    for c in range(nch):
                        kt = kpool.tile([P, cj, d], F32, tag="kt")
                        eng = getattr(nc, dma_eng)
                        kt_flat = kt[:].rearrange("p c d -> p (c d)")
                        step = cj * d // dma_split
                        for s in range(dma_split):
                            eng.dma_start(
                                out=kt_flat[:, s * step : (s + 1) * step],
                                in_=kv[:, c, s * step : (s + 1) * step],
                            )
                        if not (do_dve or do_act):
                            # keep the load live with a negligible consumer
                            nc.vector.tensor_copy(out=vdummy,
                                                  in_=kt[:, 0, 0:1])
                        for j in range(cj):
                            idx = b * t_cols + c * cj + j
                            if do_dve:
                                nc.vector.scalar_tensor_tensor(
                                    out=vdummy.broadcast_to((P, d)),
                                    in0=kt[:, j, :],
                                    scalar=1.0,
                                    in1=qrep[:, b, :],
                                    op0=Alu.mult,
                                    op1=Alu.mult,
                                    accum_out=dots[:, idx : idx + 1],
                                )
                            if do_act:
                                aout = ascr.tile([P, d], F32, tag="aout")
                                nc.scalar.activation(
                                    out=aout,
                                    in_=kt[:, j, :],
                                    func=Act.Square,
                                    accum_out=ssqs[:, idx : idx + 1],
                                )

                if epi == "batch":
                    emit_epilogue(bpc - 1)
                else:
                    for b in range(bpc):
                        emit_epilogue(b)

    nc.compile()
    return nc


_NC_CACHE = None


def _get_nc():
    global _NC_CACHE
    if _NC_CACHE is None:
        _NC_CACHE = build_nc()
    return _NC_CACHE


def kernel(query: np.ndarray, keys: np.ndarray, mask: np.ndarray) -> np.ndarray:
    assert query.shape == (B, D) and keys.shape == (B, L, D) and mask.shape == (B, L)
    from concourse.bass_utils import run_bass_kernel_spmd

    nc = _get_nc()
    mask_u8 = np.ascontiguousarray(mask).view(np.uint8)
    in_maps = []
    for i in range(N_CORES):
        sl = slice(i * BPC, (i + 1) * BPC)
        in_maps.append(
            {
                "q": np.ascontiguousarray(query[sl], dtype=np.float32),
                "keys": np.ascontiguousarray(keys[sl], dtype=np.float32),
                "mask": np.ascontiguousarray(mask_u8[sl]),
            }
        )
    res = run_bass_kernel_spmd(nc, in_maps, core_ids=list(range(N_CORES)))
    out = np.concatenate([r["out"] for r in res.results], axis=0)
    return out.astype(np.float32, copy=False)

